# revision 9
# baseline (speedup 1.0000x reference)
"""KalmanNet SLAM DNN forward pass on a single Trainium2 NeuronCore.

Network: x(14) -> Linear(560)+ReLU -> GRUCell(145) -> GRUCell(145)
         -> Linear(40)+ReLU -> Linear(10) -> reshape (5,2)

Total weights ~1.8MB fp32 => memory-bound. Strategy (per sharding hint):
replicate on one core. Host-side numpy repacks weights into the PE's
native [K-on-partitions, M-free] layout (so every DMA is contiguous),
folds biases in as an extra weight row against a constant-1.0 input
element, and pads each GRU gate block from 145 to 146 columns so all
matvec output chunks are a uniform 73 partitions.

On-chip: weights-stationary matvecs on the TensorEngine (matmul N=1,
cost is LDWEIGHTS streaming), gi+gh gate sums accumulated in PSUM,
pointwise GRU math on the VectorEngine, Sigmoid/Tanh on the ScalarE
(one table-set load, triggered early by a dummy op so it overlaps DMA).
"""

import numpy as np

import concourse.bacc as bacc
import concourse.mybir as mybir
import concourse.tile as tile
from concourse import bass_utils

F32 = mybir.dt.float32
AF = mybir.ActivationFunctionType

X_DIM, Y_DIM = 5, 2
H1, H2 = 560, 40
G = 145          # GRU hidden size
G3 = 3 * G       # 435
C = 73           # partition chunk for the GRU state (2*73 = 146 = G+1)
GP = 2 * C       # per-gate padded column block: 145 real + 1 zero col
M3 = 3 * GP      # 438 padded gate columns

TRACE = False    # test.py flips this to profile

_BUILT = None    # (nc, input_names)


def _pack_gru_w(W, b, z_pad_bias=0.0):
    """W:(3G,K) b:(3G,) -> (K+1, 438): rows=W.T plus bias row; per-gate
    146-col blocks with a zero pad col so M-chunks are uniform 73.

    z_pad_bias: bias planted in the z-gate's pad column. With 100.0 on the
    Wih matrix, the garbage slot (72,1) of h' computes to exactly
    n + sigmoid(100)*(1.0 - n) with n=tanh(0)=0, i.e. the 1.0 the next
    layer's folded-bias row needs — no extra fixup op."""
    K = W.shape[1]
    full = np.concatenate([W.T, b[None, :]], axis=0).astype(np.float32)  # (K+1, 435)
    out = np.zeros((K + 1, M3), np.float32)
    for g in range(3):
        out[:, g * GP: g * GP + G] = full[:, g * G: (g + 1) * G]
    out[K, 1 * GP + G] = z_pad_bias
    return np.ascontiguousarray(out)


def _pack_h(h):
    """(145,) -> [73, 2] col-chunk layout: [p, c] = v[c*73 + p], with the
    trailing slot (72,1) = 1.0 (the bias-row activation element)."""
    v = np.append(h.astype(np.float32), np.float32(1.0))  # (146,)
    return np.ascontiguousarray(v.reshape(2, C).T)


def _emit_gru(nc, pp, ab, name, wih_chunks, whh_chunks, h_sb, ptag_rz, ptag_ni, ptag_nh):
    """Emit one GRU cell. wih_chunks/whh_chunks: list of (lhsT_tile, rhs_ap)
    K-chunks; weight tiles have 438 padded gate columns. h_sb: [73,2] prev
    hidden (slot (72,1)=1.0). Returns h' as a [73,2] SBUF tile with
    slot (72,1) set to 1.0."""
    ps_rz = pp.tile([C, 4], F32, tag=ptag_rz)   # col 2g+c: gate g (r=0,z=1), chunk c
    ps_ni = pp.tile([C, 2], F32, tag=ptag_ni)   # Wih n-gate part (+bih_n)
    ps_nh = pp.tile([C, 2], F32, tag=ptag_nh)   # Whh n-gate part (+bhh_n)

    # One accumulation group per psum bank: start=True only on the first MM
    # into the tile, stop=True only on the last; has_written bits handle the
    # disjoint columns. gh is emitted first (PE runs in program order and
    # Whh/h arrive before Wih), gi accumulates on top for the r,z gates.
    nwh, nwi = len(whh_chunks), len(wih_chunks)
    rz_n = 4 * (nwh + nwi)
    rz_i = ni_i = nh_i = 0
    for kc, (lhsT, rhs) in enumerate(whh_chunks):
        for g in (0, 1):
            for c in (0, 1):
                nc.tensor.matmul(
                    ps_rz[:, 2 * g + c: 2 * g + c + 1],
                    lhsT[:, g * GP + C * c: g * GP + C * (c + 1)],
                    rhs, start=(rz_i == 0), stop=(rz_i == rz_n - 1))
                rz_i += 1
        for c in (0, 1):
            nc.tensor.matmul(
                ps_nh[:, c: c + 1],
                lhsT[:, 2 * GP + C * c: 2 * GP + C * (c + 1)],
                rhs, start=(nh_i == 0), stop=(nh_i == 2 * nwh - 1))
            nh_i += 1
    for kc, (lhsT, rhs) in enumerate(wih_chunks):
        for g in (0, 1):
            for c in (0, 1):
                nc.tensor.matmul(
                    ps_rz[:, 2 * g + c: 2 * g + c + 1],
                    lhsT[:, g * GP + C * c: g * GP + C * (c + 1)],
                    rhs, start=(rz_i == 0), stop=(rz_i == rz_n - 1))
                rz_i += 1
        for c in (0, 1):
            nc.tensor.matmul(
                ps_ni[:, c: c + 1],
                lhsT[:, 2 * GP + C * c: 2 * GP + C * (c + 1)],
                rhs, start=(ni_i == 0), stop=(ni_i == 2 * nwi - 1))
            ni_i += 1

    # pointwise: r,z = sigmoid(gi_rz + gh_rz); n = tanh(i_n + r*h_n);
    # h' = n + z*(h_prev - n)
    rz = ab.tile([C, 4], F32, tag=f"{name}_rz")
    nc.scalar.activation(rz, ps_rz, AF.Sigmoid)
    t1 = ab.tile([C, 2], F32, tag=f"{name}_t1")
    nc.vector.tensor_mul(t1, rz[:, 0:2], ps_nh)
    nc.vector.tensor_add(t1, t1, ps_ni)
    n_sb = ab.tile([C, 2], F32, tag=f"{name}_n")
    nc.scalar.activation(n_sb, t1, AF.Tanh)
    d = ab.tile([C, 2], F32, tag=f"{name}_d")
    nc.vector.tensor_sub(d, h_sb, n_sb)
    nc.vector.tensor_mul(d, d, rz[:, 2:4])
    hp = ab.tile([C, 2], F32, tag=f"{name}_hp")
    nc.vector.tensor_add(hp, n_sb, d)
    return hp


def _build():
    nc = bacc.Bacc("TRN2", num_devices=1)

    d_x = nc.dram_tensor("x_ext", [15, 1], F32, kind="ExternalInput").ap()
    d_w1 = nc.dram_tensor("w1t", [15, H1], F32, kind="ExternalInput").ap()
    d_wih0 = nc.dram_tensor("wih0t", [H1 + 1, M3], F32, kind="ExternalInput").ap()
    d_whh0 = nc.dram_tensor("whh0t", [G + 1, M3], F32, kind="ExternalInput").ap()
    d_wih1 = nc.dram_tensor("wih1t", [G + 1, M3], F32, kind="ExternalInput").ap()
    d_whh1 = nc.dram_tensor("whh1t", [G + 1, M3], F32, kind="ExternalInput").ap()
    d_w2a = nc.dram_tensor("w2at", [G + 1, H2], F32, kind="ExternalInput").ap()
    d_w2b = nc.dram_tensor("w2bt", [H2 + 1, X_DIM * Y_DIM], F32, kind="ExternalInput").ap()
    d_h0 = nc.dram_tensor("h0p", [C, 2], F32, kind="ExternalInput").ap()
    d_h1 = nc.dram_tensor("h1p", [C, 2], F32, kind="ExternalInput").ap()
    d_out = nc.dram_tensor("out", [1, X_DIM * Y_DIM], F32, kind="ExternalOutput").ap()

    with tile.TileContext(nc) as tc:
        with (
            tc.tile_pool(name="wp", bufs=1) as wp,
            tc.tile_pool(name="ab", bufs=1) as ab,
            tc.tile_pool(name="pp", bufs=1, space="PSUM") as pp,
        ):
            # --- ACT table warmup: pull the sigmoid/tanh table-set load
            # to t~0 so it overlaps the weight DMAs.
            warm = ab.tile([1, 1], F32, tag="warm")
            nc.vector.memset(warm, 0.0)
            warm2 = ab.tile([1, 1], F32, tag="warm2")
            nc.scalar.activation(warm2, warm, AF.Sigmoid)
            nc.scalar.activation(warm2, warm2, AF.Tanh)

            # --- DMAs, in priority order (HWDGE) ---
            x_sb = ab.tile([15, 1], F32, tag="x")
            nc.sync.dma_start(x_sb, d_x)
            w1_sb = wp.tile([15, H1], F32, tag="w1")
            nc.sync.dma_start(w1_sb, d_w1)
            h0_sb = ab.tile([C, 2], F32, tag="h0")
            nc.sync.dma_start(h0_sb, d_h0)
            h1_sb = ab.tile([C, 2], F32, tag="h1")
            nc.sync.dma_start(h1_sb, d_h1)
            whh0_sb = []
            for c in range(2):
                t = wp.tile([C, M3], F32, tag=f"whh0_{c}")
                nc.sync.dma_start(t, d_whh0[c * C:(c + 1) * C, :])
                whh0_sb.append(t)
            wih0_sb = []
            for c in range(5):
                rows = 113 if c == 4 else 112
                t = wp.tile([rows, M3], F32, tag=f"wih0_{c}")
                nc.sync.dma_start(t, d_wih0[c * 112: c * 112 + rows, :])
                wih0_sb.append(t)
            wih1_sb = []
            for c in range(2):
                t = wp.tile([C, M3], F32, tag=f"wih1_{c}")
                nc.sync.dma_start(t, d_wih1[c * C:(c + 1) * C, :])
                wih1_sb.append(t)
            whh1_sb = []
            for c in range(2):
                t = wp.tile([C, M3], F32, tag=f"whh1_{c}")
                nc.sync.dma_start(t, d_whh1[c * C:(c + 1) * C, :])
                whh1_sb.append(t)
            w2a_sb = []
            for c in range(2):
                t = wp.tile([C, H2], F32, tag=f"w2a_{c}")
                nc.sync.dma_start(t, d_w2a[c * C:(c + 1) * C, :])
                w2a_sb.append(t)
            w2b_sb = wp.tile([H2 + 1, X_DIM * Y_DIM], F32, tag="w2b")
            nc.sync.dma_start(w2b_sb, d_w2b)

            # --- layer 1: l1 = relu(W1 @ x + b1), output as [112, 5] cols ---
            ps_l1 = pp.tile([112, 5], F32, tag="p0")
            for c in range(5):
                nc.tensor.matmul(
                    ps_l1[:, c: c + 1], w1_sb[:, c * 112:(c + 1) * 112], x_sb,
                    start=(c == 0), stop=(c == 4))
            l1_sb = ab.tile([113, 5], F32, tag="l1")
            nc.vector.memset(l1_sb[:, 4:5], 1.0)   # row 112 stays 1.0 (bias slot)
            nc.vector.tensor_scalar_max(l1_sb[0:112, :], ps_l1, 0.0)

            # --- GRU 0 ---
            wih0_chunks = [
                (wih0_sb[c], l1_sb[0:(113 if c == 4 else 112), c: c + 1])
                for c in range(5)
            ]
            whh0_chunks = [(whh0_sb[c], h0_sb[:, c: c + 1]) for c in range(2)]
            hp0 = _emit_gru(nc, pp, ab, "g0", wih0_chunks, whh0_chunks, h0_sb,
                            "p1", "p2", "p3")

            # --- GRU 1 ---
            wih1_chunks = [(wih1_sb[c], hp0[:, c: c + 1]) for c in range(2)]
            whh1_chunks = [(whh1_sb[c], h1_sb[:, c: c + 1]) for c in range(2)]
            hp1 = _emit_gru(nc, pp, ab, "g1", wih1_chunks, whh1_chunks, h1_sb,
                            "p0", "p1", "p2")

            # --- l2: relu(W2a @ h1 + b2a) then W2b @ . + b2b ---
            ps_a = pp.tile([H2, 1], F32, tag="p3")
            for c in range(2):
                nc.tensor.matmul(ps_a, w2a_sb[c], hp1[:, c: c + 1],
                                 start=(c == 0), stop=(c == 1))
            l2h = ab.tile([H2 + 1, 1], F32, tag="l2h")
            nc.vector.memset(l2h, 1.0)             # row 40 stays 1.0 (bias slot)
            nc.vector.tensor_scalar_max(l2h[0:H2, :], ps_a, 0.0)
            ps_o = pp.tile([1, X_DIM * Y_DIM], F32, tag="p4")
            nc.tensor.matmul(ps_o, l2h, w2b_sb, start=True, stop=True)
            out_sb = ab.tile([1, X_DIM * Y_DIM], F32, tag="out_sb")
            nc.vector.tensor_copy(out_sb, ps_o)
            nc.sync.dma_start(d_out, out_sb)

    nc.compile()
    return nc


def _get_nc():
    global _BUILT
    if _BUILT is None:
        _BUILT = _build()
    return _BUILT


def pack_inputs(inputs):
    """Host-side repack of the raw nn inputs into the kernel's DRAM tensors."""
    f = lambda a: np.asarray(a, np.float32)
    x_ext = np.concatenate([
        f(inputs["state_inno"]), f(inputs["obs_inno"]),
        f(inputs["diff_state"]), f(inputs["diff_obs"]), [np.float32(1.0)],
    ]).astype(np.float32).reshape(15, 1)
    w1t = np.ascontiguousarray(
        np.concatenate([f(inputs["W1"]).T, f(inputs["b1"])[None, :]], axis=0))
    hn = f(inputs["hn"])
    return {
        "x_ext": x_ext,
        "w1t": w1t,
        "wih0t": _pack_gru_w(f(inputs["Wih0"]), f(inputs["bih0"]), z_pad_bias=100.0),
        "whh0t": _pack_gru_w(f(inputs["Whh0"]), f(inputs["bhh0"])),
        "wih1t": _pack_gru_w(f(inputs["Wih1"]), f(inputs["bih1"]), z_pad_bias=100.0),
        "whh1t": _pack_gru_w(f(inputs["Whh1"]), f(inputs["bhh1"])),
        "w2at": np.ascontiguousarray(
            np.concatenate([f(inputs["W2a"]).T, f(inputs["b2a"])[None, :]], axis=0)),
        "w2bt": np.ascontiguousarray(
            np.concatenate([f(inputs["W2b"]).T, f(inputs["b2b"])[None, :]], axis=0)),
        "h0p": _pack_h(hn[0]),
        "h1p": _pack_h(hn[1]),
    }


def kernel(**inputs):
    nc = _get_nc()
    in_map = pack_inputs(inputs)
    res = bass_utils.run_bass_kernel_spmd(nc, [in_map], core_ids=[0], trace=TRACE)
    kernel.last_result = res
    out = np.asarray(res.results[0]["out"], np.float32).reshape(X_DIM, Y_DIM)
    return out


# revision 11
# speedup vs baseline: 1.2788x; 1.2788x over previous
"""KalmanNet SLAM DNN forward pass on a single Trainium2 NeuronCore.

Network: x(14) -> Linear(560)+ReLU -> GRUCell(145) -> GRUCell(145)
         -> Linear(40)+ReLU -> Linear(10) -> reshape (5,2)

~1.8MB of fp32 weights, single sample => memory-bound, replicate on one
core (per sharding hint). All math is kept exact fp32.

Host-side numpy packs everything into three partition-major DRAM images
(so each SWDGE DMA moves ~10-14KB per partition in one descriptor, near
line rate), with weights transposed to the PE's [K-on-partitions, M-free]
layout, biases folded in as an extra weight row against a constant-1.0
input element, and each GRU gate block padded 145->146 columns so all
matvec output chunks are a uniform 73 partitions.

On-chip: weights-stationary fp32 matvecs on the TensorEngine (N=1
matmuls; gi+gh gate sums accumulate in PSUM), pointwise GRU math on the
VectorEngine (with per-partition-scalar fusion for i_n + r*h_n),
Sigmoid/Tanh on ScalarE. A dummy Sigmoid/Tanh at t=0 pulls the ACT
table-set load into the DMA window, and a burst of dummy matmuls warms
the PE clock (HAM) before the real matvecs arrive.
"""

import numpy as np

import concourse.bacc as bacc
import concourse.mybir as mybir
import concourse.tile as tile
from concourse import bass_utils

F32 = mybir.dt.float32
AF = mybir.ActivationFunctionType
ALU = mybir.AluOpType

X_DIM, Y_DIM = 5, 2
H1, H2 = 560, 40
G = 145          # GRU hidden size
C = 73           # partition chunk for the GRU state (2*73 = 146 = G+1)
GP = 2 * C       # per-gate padded column block: 145 real + 1 zero col
M3 = 3 * GP      # 438 padded gate columns

# megaB column layout (73-partition image)
B_H0, B_H1 = 0, 2
B_WHH0, B_WIH1, B_WHH1 = 4, 4 + M3 * 2, 4 + M3 * 4
B_W2A, B_W2B = 4 + M3 * 6, 4 + M3 * 6 + 2 * H2
B_F = B_W2B + X_DIM * Y_DIM          # 2722 cols

A_F = 5 * M3                          # megaA: wih0, 5 K-chunks of 438 cols
C_F = H1 + 1                          # megaC: W1T (560) + x_ext col

TRACE = False    # test.py flips this to profile
_BUILT = None


def _emit_gru(nc, pp, ab, name, wih_chunks, whh_chunks, h_sb, ptag_rz, ptag_ni, ptag_nh):
    """One GRU cell. *_chunks: (lhsT_ap[K, 438], rhs_ap[K, 1]) K-chunks.
    h_sb: [73,2] prev hidden, slot (72,1)=1.0. Returns h' [73,2] with
    slot (72,1) computing to exactly 1.0 (z-gate pad-bias trick)."""
    ps_rz = pp.tile([C, 4], F32, tag=ptag_rz)
    ps_ni = pp.tile([C, 2], F32, tag=ptag_ni)
    ps_nh = pp.tile([C, 2], F32, tag=ptag_nh)

    nwh, nwi = len(whh_chunks), len(wih_chunks)
    rz_n = 4 * (nwh + nwi)
    rz_i = ni_i = nh_i = 0
    for kc, (lhsT, rhs) in enumerate(whh_chunks):
        for g in (0, 1):
            for c in (0, 1):
                nc.tensor.matmul(
                    ps_rz[:, 2 * g + c: 2 * g + c + 1],
                    lhsT[:, g * GP + C * c: g * GP + C * (c + 1)],
                    rhs, start=(rz_i == 0), stop=(rz_i == rz_n - 1))
                rz_i += 1
        for c in (0, 1):
            nc.tensor.matmul(
                ps_nh[:, c: c + 1],
                lhsT[:, 2 * GP + C * c: 2 * GP + C * (c + 1)],
                rhs, start=(nh_i == 0), stop=(nh_i == 2 * nwh - 1))
            nh_i += 1
    for kc, (lhsT, rhs) in enumerate(wih_chunks):
        last = kc == nwi - 1
        for g in (0, 1):
            for c in (0, 1):
                nc.tensor.matmul(
                    ps_rz[:, 2 * g + c: 2 * g + c + 1],
                    lhsT[:, g * GP + C * c: g * GP + C * (c + 1)],
                    rhs, start=(rz_i == 0), stop=(rz_i == rz_n - 1))
                rz_i += 1
        for c in (0, 1):
            nc.tensor.matmul(
                ps_ni[:, c: c + 1],
                lhsT[:, 2 * GP + C * c: 2 * GP + C * (c + 1)],
                rhs, start=(ni_i == 0), stop=(ni_i == 2 * nwi - 1))
            ni_i += 1

    # r,z = sigmoid(rz-sums); n = tanh(i_n + r*h_n); h' = n + z*(h-n)
    rz = ab.tile([C, 4], F32, tag=f"{name}_rz")
    nc.scalar.activation(rz, ps_rz, AF.Sigmoid)
    t1 = ab.tile([C, 2], F32, tag=f"{name}_t1")
    nc.vector.tensor_mul(t1, rz[:, 0:2], ps_nh)
    nc.vector.tensor_add(t1, t1, ps_ni)
    n_sb = ab.tile([C, 2], F32, tag=f"{name}_n")
    nc.scalar.activation(n_sb, t1, AF.Tanh)
    d = ab.tile([C, 2], F32, tag=f"{name}_d")
    nc.vector.tensor_sub(d, h_sb, n_sb)
    nc.vector.tensor_mul(d, d, rz[:, 2:4])
    hp = ab.tile([C, 2], F32, tag=f"{name}_hp")
    nc.vector.tensor_add(hp, n_sb, d)
    return hp


def _build():
    nc = bacc.Bacc("TRN2", num_devices=1)

    d_a = nc.dram_tensor("mega_a", [113, A_F], F32, kind="ExternalInput").ap()
    d_b = nc.dram_tensor("mega_b", [C, B_F], F32, kind="ExternalInput").ap()
    d_c = nc.dram_tensor("mega_c", [15, C_F], F32, kind="ExternalInput").ap()
    d_out = nc.dram_tensor("out", [1, X_DIM * Y_DIM], F32, kind="ExternalOutput").ap()

    with tile.TileContext(nc) as tc:
        with (
            tc.tile_pool(name="wp", bufs=1) as wp,
            tc.tile_pool(name="ab", bufs=1) as ab,
            tc.tile_pool(name="pp", bufs=1, space="PSUM") as pp,
        ):
            # ACT table warmup (sigmoid/tanh set loads during DMA window)
            warm = ab.tile([1, 1], F32, tag="warm")
            nc.vector.memset(warm, 0.0)
            warm2 = ab.tile([1, 1], F32, tag="warm2")
            nc.scalar.activation(warm2, warm, AF.Sigmoid)
            nc.scalar.activation(warm2, warm2, AF.Tanh)

            # --- DMAs: SWDGE, partition-major images, in priority order ---
            mc = wp.tile([15, C_F], F32, tag="mc")
            nc.gpsimd.dma_start(mc, d_c)
            mb = wp.tile([C, B_F], F32, tag="mb")
            nc.gpsimd.dma_start(mb[:, 0:B_WIH1], d_b[:, 0:B_WIH1])
            ma = wp.tile([113, A_F], F32, tag="ma")
            for s in range(3):  # wih0 in 3 pieces for DMA/PE pipelining
                lo = s * 2 * M3
                hi = min(A_F, lo + 2 * M3)
                nc.gpsimd.dma_start(ma[:, lo:hi], d_a[:, lo:hi])
            nc.gpsimd.dma_start(mb[:, B_WIH1:B_WHH1], d_b[:, B_WIH1:B_WHH1])
            nc.gpsimd.dma_start(mb[:, B_WHH1:B_F], d_b[:, B_WHH1:B_F])

            # --- PE warmup: ~3.5us of dummy fp32 matmuls (HAM) ---
            wz = ab.tile([128, 128], F32, tag="wz")
            nc.vector.memset(wz, 0.0)
            ps_w = pp.tile([2, 128], F32, tag="pw")
            for i in range(8):
                nc.tensor.matmul(ps_w, wz[:, 0:2], wz,
                                 start=(i == 0), stop=(i == 7))

            # --- layer 1: l1 = relu(W1 @ x + b1) as [112, 5] columns ---
            x_sb = mc[:, H1:H1 + 1]                       # [15, 1]
            ps_l1 = pp.tile([112, 5], F32, tag="p0")
            for c in range(5):
                nc.tensor.matmul(
                    ps_l1[:, c: c + 1], mc[:, c * 112:(c + 1) * 112], x_sb,
                    start=(c == 0), stop=(c == 4))
            l1_sb = ab.tile([113, 5], F32, tag="l1")
            nc.vector.memset(l1_sb[:, 4:5], 1.0)          # row 112 = bias slot
            nc.vector.tensor_scalar_max(l1_sb[0:112, :], ps_l1, 0.0)

            h0_sb = mb[:, B_H0:B_H0 + 2]
            h1_sb = mb[:, B_H1:B_H1 + 2]

            # --- GRU 0 ---
            wih0_chunks = [
                (ma[0:(113 if c == 4 else 112), c * M3:(c + 1) * M3],
                 l1_sb[0:(113 if c == 4 else 112), c: c + 1])
                for c in range(5)
            ]
            whh0_chunks = [
                (mb[:, B_WHH0 + c * M3: B_WHH0 + (c + 1) * M3], h0_sb[:, c: c + 1])
                for c in range(2)
            ]
            hp0 = _emit_gru(nc, pp, ab, "g0", wih0_chunks, whh0_chunks, h0_sb,
                            "p1", "p2", "p3")

            # --- GRU 1 ---
            wih1_chunks = [
                (mb[:, B_WIH1 + c * M3: B_WIH1 + (c + 1) * M3], hp0[:, c: c + 1])
                for c in range(2)
            ]
            whh1_chunks = [
                (mb[:, B_WHH1 + c * M3: B_WHH1 + (c + 1) * M3], h1_sb[:, c: c + 1])
                for c in range(2)
            ]
            hp1 = _emit_gru(nc, pp, ab, "g1", wih1_chunks, whh1_chunks, h1_sb,
                            "p0", "p1", "p2")

            # --- l2: relu(W2a @ h1' + b2a) then row-out W2b matvec ---
            ps_a = pp.tile([H2, 1], F32, tag="p3")
            for c in range(2):
                nc.tensor.matmul(
                    ps_a, mb[:, B_W2A + c * H2: B_W2A + (c + 1) * H2],
                    hp1[:, c: c + 1], start=(c == 0), stop=(c == 1))
            l2h = ab.tile([H2 + 1, 1], F32, tag="l2h")
            nc.vector.memset(l2h, 1.0)                    # row 40 = bias slot
            nc.vector.tensor_scalar_max(l2h[0:H2, :], ps_a, 0.0)
            ps_o = pp.tile([1, X_DIM * Y_DIM], F32, tag="p4")
            nc.tensor.matmul(ps_o, l2h, mb[0:H2 + 1, B_W2B:B_W2B + X_DIM * Y_DIM],
                             start=True, stop=True)
            out_sb = ab.tile([1, X_DIM * Y_DIM], F32, tag="out_sb")
            nc.vector.tensor_copy(out_sb, ps_o)
            nc.sync.dma_start(d_out, out_sb)

    nc.compile()
    return nc


def _get_nc():
    global _BUILT
    if _BUILT is None:
        _BUILT = _build()
    return _BUILT


def _gate_pack(W, b, z_pad_bias=0.0):
    """W:(435,K), b:(435,) -> (K+1, 438) W.T+bias-row, gates padded to 146
    cols. z_pad_bias=100 makes downstream h'-slot (72,1) compute to 1.0."""
    K = W.shape[1]
    full = np.concatenate([W.T, b[None, :]], axis=0).astype(np.float32)
    out = np.zeros((K + 1, M3), np.float32)
    for g in range(3):
        out[:, g * GP: g * GP + G] = full[:, g * G: (g + 1) * G]
    out[K, GP + G] = z_pad_bias
    return out


def pack_inputs(inputs):
    f = lambda a: np.asarray(a, np.float32)
    # megaA [113, 2190]: wih0 K-chunks (4x112 + 113 rows)
    wih0 = _gate_pack(f(inputs["Wih0"]), f(inputs["bih0"]), z_pad_bias=100.0)  # (561, 438)
    ma = np.zeros((113, A_F), np.float32)
    for c in range(5):
        rows = 113 if c == 4 else 112
        ma[0:rows, c * M3:(c + 1) * M3] = wih0[c * 112: c * 112 + rows, :]

    # megaB [73, 2722]
    mb = np.zeros((C, B_F), np.float32)
    hn = f(inputs["hn"])
    for col, h in ((B_H0, hn[0]), (B_H1, hn[1])):
        v = np.append(h.astype(np.float32), np.float32(1.0))
        mb[:, col:col + 2] = v.reshape(2, C).T
    for col, W, b, zb in (
        (B_WHH0, inputs["Whh0"], inputs["bhh0"], 0.0),
        (B_WIH1, inputs["Wih1"], inputs["bih1"], 100.0),
        (B_WHH1, inputs["Whh1"], inputs["bhh1"], 0.0),
    ):
        wt = _gate_pack(f(W), f(b), zb)                   # (146, 438)
        mb[:, col:col + M3] = wt[0:C, :]
        mb[:, col + M3:col + 2 * M3] = wt[C:2 * C, :]
    w2a = np.concatenate([f(inputs["W2a"]).T, f(inputs["b2a"])[None, :]], axis=0)  # (146, 40)
    mb[:, B_W2A:B_W2A + H2] = w2a[0:C, :]
    mb[:, B_W2A + H2:B_W2A + 2 * H2] = w2a[C:2 * C, :]
    w2b = np.concatenate([f(inputs["W2b"]).T, f(inputs["b2b"])[None, :]], axis=0)  # (41, 10)
    mb[0:H2 + 1, B_W2B:B_W2B + X_DIM * Y_DIM] = w2b

    # megaC [15, 561]: W1T + bias row, then x_ext column
    mc = np.zeros((15, C_F), np.float32)
    mc[:, 0:H1] = np.concatenate(
        [f(inputs["W1"]).T, f(inputs["b1"])[None, :]], axis=0)
    mc[:, H1] = np.concatenate([
        f(inputs["state_inno"]), f(inputs["obs_inno"]),
        f(inputs["diff_state"]), f(inputs["diff_obs"]), [np.float32(1.0)],
    ])
    return {"mega_a": ma, "mega_b": mb, "mega_c": mc}


def kernel(**inputs):
    nc = _get_nc()
    in_map = pack_inputs(inputs)
    res = bass_utils.run_bass_kernel_spmd(nc, [in_map], core_ids=[0], trace=TRACE)
    kernel.last_result = res
    return np.asarray(res.results[0]["out"], np.float32).reshape(X_DIM, Y_DIM)


# revision 15
# speedup vs baseline: 1.4725x; 1.1515x over previous
"""KalmanNet SLAM DNN forward pass on a single Trainium2 NeuronCore.

Network: x(14) -> Linear(560)+ReLU -> GRUCell(145) -> GRUCell(145)
         -> Linear(40)+ReLU -> Linear(10) -> reshape (5,2)

~1.8MB of fp32 weights, single sample => memory-bound; replicate on one
core (per sharding hint).

Matvecs run weights-stationary on the TensorEngine in fp32r (single-pass
fp32; ~1e-4-class relative error, far inside the scale-relative gate).
fp32r requires an even moving free dim, so every activation vector is
kept in duplicated column pairs ([K,2] rhs -> [M,2] psum) end to end.

Host-side numpy packs everything into three partition-major DRAM images
(per-partition contiguous runs of 2-14KB => near-line-rate SWDGE
descriptors, ~5% padding total), weights pre-transposed to [K, M]
layout, biases folded as an extra weight row against a constant-1.0
input element, GRU gates padded 145->146 so output chunks are uniform
73 partitions, and the z-gate pad-column bias set to 100 so the h'
garbage slot computes to exactly the 1.0 the next bias row needs.

Pointwise GRU math on VectorE, Sigmoid/Tanh on ScalarE (table-set load
pulled to t=0 by a dummy op), plus a dummy-matmul burst to warm the PE
clock during the DMA window.
"""

import numpy as np

import concourse.bacc as bacc
import concourse.mybir as mybir
import concourse.tile as tile
from concourse import bass_utils

F32 = mybir.dt.float32
F32R = mybir.dt.float32r
AF = mybir.ActivationFunctionType

X_DIM, Y_DIM = 5, 2
H1, H2 = 560, 40
G = 145          # GRU hidden size
C = 73           # partition chunk for the GRU state (2*73 = 146 = G+1)
GP = 2 * C       # per-gate padded column block
M3 = 3 * GP      # 438 padded gate columns
NO = X_DIM * Y_DIM

# megaB (73-partition image) column map
B_H0, B_H1 = 0, 4                      # h pairs [73,4]: (c0,c0,c1,c1)
B_WHH0 = 8                             # 2 x 438
B_WIH0C4 = B_WHH0 + 2 * M3             # [49,438]
B_WIH1 = B_WIH0C4 + M3                 # 2 x 438
B_WHH1 = B_WIH1 + 2 * M3               # 2 x 438
B_W2A = B_WHH1 + 2 * M3                # 2 x 41 (41st col makes the 1.0)
B_W2B = B_W2A + 2 * (H2 + 1)           # [41,10]
B_F = B_W2B + NO                       # 3164

A_F = 4 * M3                           # mega128: wih0 chunks c0..c3
C_F = H1 + 3                           # megaC: W1T(561, unit col) + x pair

TRACE = False
_BUILT = None


def _emit_gru(nc, pp, ab, name, wih_chunks, whh_chunks, h_sb,
              ptag_rz, ptag_ni, ptag_nh):
    """One GRU cell, everything in duplicated column pairs.
    *_chunks: (lhsT[K, 438], rhs[K, 2]); h_sb: [73,4] prev hidden pairs
    with slots (72, 2:4) = 1.0. Returns h' [73,4] F32R pairs."""
    ps_rz = pp.tile([C, 8], F32, tag=ptag_rz)   # (r_c0, r_c1, z_c0, z_c1) pairs
    ps_ni = pp.tile([C, 4], F32, tag=ptag_ni)
    ps_nh = pp.tile([C, 4], F32, tag=ptag_nh)

    nwi, nwh = len(wih_chunks), len(whh_chunks)
    rz_n = 4 * (nwh + nwi)
    rz_i = ni_i = nh_i = 0
    # gi first (its weights arrive first), gh accumulates on top
    for kc, (lhsT, rhs) in enumerate(wih_chunks):
        for g in (0, 1):
            for c in (0, 1):
                j = 2 * g + c
                nc.tensor.matmul(
                    ps_rz[:, 2 * j: 2 * j + 2],
                    lhsT[:, g * GP + C * c: g * GP + C * (c + 1)],
                    rhs, start=(rz_i == 0), stop=(rz_i == rz_n - 1))
                rz_i += 1
        for c in (0, 1):
            nc.tensor.matmul(
                ps_ni[:, 2 * c: 2 * c + 2],
                lhsT[:, 2 * GP + C * c: 2 * GP + C * (c + 1)],
                rhs, start=(ni_i == 0), stop=(ni_i == 2 * nwi - 1))
            ni_i += 1
    for kc, (lhsT, rhs) in enumerate(whh_chunks):
        for g in (0, 1):
            for c in (0, 1):
                j = 2 * g + c
                nc.tensor.matmul(
                    ps_rz[:, 2 * j: 2 * j + 2],
                    lhsT[:, g * GP + C * c: g * GP + C * (c + 1)],
                    rhs, start=(rz_i == 0), stop=(rz_i == rz_n - 1))
                rz_i += 1
        for c in (0, 1):
            nc.tensor.matmul(
                ps_nh[:, 2 * c: 2 * c + 2],
                lhsT[:, 2 * GP + C * c: 2 * GP + C * (c + 1)],
                rhs, start=(nh_i == 0), stop=(nh_i == 2 * nwh - 1))
            nh_i += 1

    # r,z = sigmoid(rz sums); n = tanh(i_n + r*h_n); h' = n + z*(h-n)
    rz = ab.tile([C, 8], F32, tag=f"{name}_rz")
    nc.scalar.activation(rz, ps_rz, AF.Sigmoid)
    t1 = ab.tile([C, 4], F32, tag=f"{name}_t1")
    nc.vector.tensor_mul(t1, rz[:, 0:4], ps_nh)
    nc.vector.tensor_add(t1, t1, ps_ni)
    n_sb = ab.tile([C, 4], F32, tag=f"{name}_n")
    nc.scalar.activation(n_sb, t1, AF.Tanh)
    d = ab.tile([C, 4], F32, tag=f"{name}_d")
    nc.vector.tensor_sub(d, h_sb.bitcast(F32), n_sb)
    nc.vector.tensor_mul(d, d, rz[:, 4:8])
    hp = ab.tile([C, 4], F32R, tag=f"{name}_hp")
    nc.vector.tensor_add(hp, n_sb, d)
    return hp


def _build():
    nc = bacc.Bacc("TRN2", num_devices=1)

    d_a = nc.dram_tensor("mega_a", [128, A_F], F32R, kind="ExternalInput").ap()
    d_b = nc.dram_tensor("mega_b", [C, B_F], F32R, kind="ExternalInput").ap()
    d_c = nc.dram_tensor("mega_c", [15, C_F], F32R, kind="ExternalInput").ap()
    d_out = nc.dram_tensor("out", [1, NO], F32, kind="ExternalOutput").ap()

    with tile.TileContext(nc) as tc:
        with (
            tc.tile_pool(name="wp", bufs=1) as wp,
            tc.tile_pool(name="ab", bufs=1) as ab,
            tc.tile_pool(name="pp", bufs=1, space="PSUM") as pp,
        ):
            # ACT table warmup
            warm = ab.tile([1, 1], F32, tag="warm")
            nc.vector.memset(warm, 0.0)
            warm2 = ab.tile([1, 1], F32, tag="warm2")
            nc.scalar.activation(warm2, warm, AF.Sigmoid)
            nc.scalar.activation(warm2, warm2, AF.Tanh)

            # --- DMAs (SWDGE), priority order ---
            mc = wp.tile([15, C_F], F32R, tag="mc")
            nc.gpsimd.dma_start(mc, d_c)
            ma = wp.tile([128, A_F], F32R, tag="ma")
            nc.gpsimd.dma_start(ma[:, 0:2 * M3], d_a[:, 0:2 * M3])
            nc.gpsimd.dma_start(ma[:, 2 * M3:A_F], d_a[:, 2 * M3:A_F])
            mb = wp.tile([C, B_F], F32R, tag="mb")
            nc.gpsimd.dma_start(mb[:, 0:B_WIH1], d_b[:, 0:B_WIH1])
            nc.gpsimd.dma_start(mb[:, B_WIH1:B_F], d_b[:, B_WIH1:B_F])

            # --- PE warmup: dummy fp32 matmuls (~3.5us of HAM activity) ---
            wz = ab.tile([128, 128], F32, tag="wz")
            nc.vector.memset(wz, 0.0)
            ps_w = pp.tile([2, 128], F32, tag="pw")
            for i in range(8):
                nc.tensor.matmul(ps_w, wz[:, 0:2], wz,
                                 start=(i == 0), stop=(i == 7))

            # --- layer 1: l1 = relu(W1 @ x + b1), [128,10] paired cols ---
            x2 = mc[:, H1 + 1:H1 + 3]
            ps_l1 = pp.tile([128, 8], F32, tag="p0")
            for c in range(4):
                nc.tensor.matmul(ps_l1[:, 2 * c:2 * c + 2],
                                 mc[:, c * 128:(c + 1) * 128], x2,
                                 start=(c == 0), stop=(c == 3))
            ps_l1b = pp.tile([49, 2], F32, tag="p5")
            nc.tensor.matmul(ps_l1b, mc[:, 512:561], x2, start=True, stop=True)
            l1_sb = ab.tile([128, 10], F32R, tag="l1")
            nc.vector.tensor_scalar_max(l1_sb[:, 0:8], ps_l1, 0.0)
            nc.vector.tensor_scalar_max(l1_sb[0:49, 8:10], ps_l1b, 0.0)

            h0_sb = mb[:, B_H0:B_H0 + 4]
            h1_sb = mb[:, B_H1:B_H1 + 4]

            # --- GRU 0 ---
            wih0_chunks = [
                (ma[:, c * M3:(c + 1) * M3], l1_sb[:, 2 * c:2 * c + 2])
                for c in range(4)
            ] + [
                (mb[0:49, B_WIH0C4:B_WIH0C4 + M3], l1_sb[0:49, 8:10])
            ]
            whh0_chunks = [
                (mb[:, B_WHH0 + c * M3: B_WHH0 + (c + 1) * M3],
                 h0_sb[:, 2 * c:2 * c + 2])
                for c in range(2)
            ]
            hp0 = _emit_gru(nc, pp, ab, "g0", wih0_chunks, whh0_chunks, h0_sb,
                            "p1", "p2", "p3")

            # --- GRU 1 ---
            wih1_chunks = [
                (mb[:, B_WIH1 + c * M3: B_WIH1 + (c + 1) * M3],
                 hp0[:, 2 * c:2 * c + 2])
                for c in range(2)
            ]
            whh1_chunks = [
                (mb[:, B_WHH1 + c * M3: B_WHH1 + (c + 1) * M3],
                 h1_sb[:, 2 * c:2 * c + 2])
                for c in range(2)
            ]
            hp1 = _emit_gru(nc, pp, ab, "g1", wih1_chunks, whh1_chunks, h1_sb,
                            "p0", "p1", "p2")

            # --- l2 ---
            ps_a = pp.tile([H2 + 1, 2], F32, tag="p3")
            for c in range(2):
                nc.tensor.matmul(
                    ps_a, mb[:, B_W2A + c * (H2 + 1): B_W2A + (c + 1) * (H2 + 1)],
                    hp1[:, 2 * c:2 * c + 2], start=(c == 0), stop=(c == 1))
            l2h = ab.tile([H2 + 1, 2], F32R, tag="l2h")
            nc.vector.tensor_scalar_max(l2h, ps_a, 0.0)
            ps_o = pp.tile([1, NO], F32, tag="p4")
            nc.tensor.matmul(ps_o, l2h[:, 0:1],
                             mb[0:H2 + 1, B_W2B:B_W2B + NO],
                             start=True, stop=True)
            out_sb = ab.tile([1, NO], F32, tag="out_sb")
            nc.vector.tensor_copy(out_sb, ps_o)
            nc.sync.dma_start(d_out, out_sb)

    nc.compile()
    return nc


def _get_nc():
    global _BUILT
    if _BUILT is None:
        _BUILT = _build()
    return _BUILT


def _gate_pack(W, b, z_pad_bias=0.0):
    """W:(435,K), b:(435,) -> (K+1, 438): W.T + bias row, per-gate 146-col
    blocks (zero pad col). z_pad_bias=100 on the ih matrix makes the h'
    garbage slot compute to exactly 1.0."""
    K = W.shape[1]
    full = np.concatenate([W.T, b[None, :]], axis=0).astype(np.float32)
    out = np.zeros((K + 1, M3), np.float32)
    for g in range(3):
        out[:, g * GP: g * GP + G] = full[:, g * G: (g + 1) * G]
    out[K, GP + G] = z_pad_bias
    return out


def pack_inputs(inputs):
    f = lambda a: np.asarray(a, np.float32)
    wih0 = _gate_pack(f(inputs["Wih0"]), f(inputs["bih0"]), 100.0)  # (561, 438)
    ma = np.zeros((128, A_F), np.float32)
    for c in range(4):
        ma[:, c * M3:(c + 1) * M3] = wih0[c * 128:(c + 1) * 128, :]

    mb = np.zeros((C, B_F), np.float32)
    hn = f(inputs["hn"])
    for col, h in ((B_H0, hn[0]), (B_H1, hn[1])):
        v = np.append(h, np.float32(1.0)).reshape(2, C).T  # [73,2]
        mb[:, col:col + 4] = v[:, [0, 0, 1, 1]]            # paired
    mb[0:49, B_WIH0C4:B_WIH0C4 + M3] = wih0[512:561, :]
    for col, W, b, zb in (
        (B_WHH0, inputs["Whh0"], inputs["bhh0"], 0.0),
        (B_WIH1, inputs["Wih1"], inputs["bih1"], 100.0),
        (B_WHH1, inputs["Whh1"], inputs["bhh1"], 0.0),
    ):
        wt = _gate_pack(f(W), f(b), zb)                    # (146, 438)
        mb[:, col:col + M3] = wt[0:C, :]
        mb[:, col + M3:col + 2 * M3] = wt[C:2 * C, :]
    w2a = np.zeros((2 * C, H2 + 1), np.float32)
    w2a[0:G + 1, 0:H2] = np.concatenate(
        [f(inputs["W2a"]).T, f(inputs["b2a"])[None, :]], axis=0)
    w2a[G, H2] = 1.0                 # unit col -> l2h slot computes to 1.0
    mb[:, B_W2A:B_W2A + H2 + 1] = w2a[0:C, :]
    mb[:, B_W2A + H2 + 1:B_W2A + 2 * (H2 + 1)] = w2a[C:2 * C, :]
    w2b = np.concatenate([f(inputs["W2b"]).T, f(inputs["b2b"])[None, :]], axis=0)
    mb[0:H2 + 1, B_W2B:B_W2B + NO] = w2b

    mc = np.zeros((15, C_F), np.float32)
    mc[:, 0:H1] = np.concatenate(
        [f(inputs["W1"]).T, f(inputs["b1"])[None, :]], axis=0)
    mc[14, H1] = 1.0                 # unit col -> l1 slot computes to 1.0
    x_ext = np.concatenate([
        f(inputs["state_inno"]), f(inputs["obs_inno"]),
        f(inputs["diff_state"]), f(inputs["diff_obs"]), [np.float32(1.0)],
    ])
    mc[:, H1 + 1] = x_ext
    mc[:, H1 + 2] = x_ext
    return {"mega_a": ma, "mega_b": mb, "mega_c": mc}


def kernel(**inputs):
    nc = _get_nc()
    in_map = pack_inputs(inputs)
    res = bass_utils.run_bass_kernel_spmd(nc, [in_map], core_ids=[0], trace=TRACE)
    kernel.last_result = res
    return np.asarray(res.results[0]["out"], np.float32).reshape(X_DIM, Y_DIM)


# revision 16
# speedup vs baseline: 1.4837x; 1.0076x over previous
"""KalmanNet SLAM DNN forward pass on a single Trainium2 NeuronCore.

Network: x(14) -> Linear(560)+ReLU -> GRUCell(145) -> GRUCell(145)
         -> Linear(40)+ReLU -> Linear(10) -> reshape (5,2)

~1.8MB of fp32 weights, single sample => memory-bound; replicate on one
core (per sharding hint).

Matvecs run weights-stationary on the TensorEngine in fp32r (single-pass
fp32; ~1e-4-class relative error, far inside the scale-relative gate).
fp32r requires an even moving free dim, so every activation vector is
kept in duplicated column pairs ([K,2] rhs -> [M,2] psum) end to end.

Host-side numpy packs everything into three partition-major DRAM images
(per-partition contiguous runs of 2-14KB => near-line-rate SWDGE
descriptors, ~5% padding total), weights pre-transposed to [K, M]
layout, biases folded as an extra weight row against a constant-1.0
input element, GRU gates padded 145->146 so output chunks are uniform
73 partitions, and the z-gate pad-column bias set to 100 so the h'
garbage slot computes to exactly the 1.0 the next bias row needs.

Pointwise GRU math on VectorE, Sigmoid/Tanh on ScalarE (table-set load
pulled to t=0 by a dummy op), plus a dummy-matmul burst to warm the PE
clock during the DMA window.
"""

import numpy as np

import concourse.bacc as bacc
import concourse.mybir as mybir
import concourse.tile as tile
from concourse import bass_utils

F32 = mybir.dt.float32
F32R = mybir.dt.float32r
AF = mybir.ActivationFunctionType

X_DIM, Y_DIM = 5, 2
H1, H2 = 560, 40
G = 145          # GRU hidden size
C = 73           # partition chunk for the GRU state (2*73 = 146 = G+1)
GP = 2 * C       # per-gate padded column block
M3 = 3 * GP      # 438 padded gate columns
NO = X_DIM * Y_DIM

# megaB (73-partition image) column map
B_H0, B_H1 = 0, 4                      # h pairs [73,4]: (c0,c0,c1,c1)
B_WHH0 = 8                             # 2 x 438
B_WIH0C4 = B_WHH0 + 2 * M3             # [49,438]
B_WIH1 = B_WIH0C4 + M3                 # 2 x 438
B_WHH1 = B_WIH1 + 2 * M3               # 2 x 438
B_W2A = B_WHH1 + 2 * M3                # 2 x 41 (41st col makes the 1.0)
B_W2B = B_W2A + 2 * (H2 + 1)           # [41,10]
B_F = B_W2B + NO                       # 3164

A_F = 4 * M3                           # mega128: wih0 chunks c0..c3
C_F = H1 + 3                           # megaC: W1T(561, unit col) + x pair

TRACE = False
_BUILT = None


def _emit_gru(nc, pp, ab, name, wih_chunks, whh_chunks, h_sb,
              ptag_rz, ptag_ni, ptag_nh):
    """One GRU cell, everything in duplicated column pairs.
    *_chunks: (lhsT[K, 438], rhs[K, 2]); h_sb: [73,4] prev hidden pairs
    with slots (72, 2:4) = 1.0. Returns h' [73,4] F32R pairs."""
    ps_rz = pp.tile([C, 8], F32, tag=ptag_rz)   # (r_c0, r_c1, z_c0, z_c1) pairs
    ps_ni = pp.tile([C, 4], F32, tag=ptag_ni)
    ps_nh = pp.tile([C, 4], F32, tag=ptag_nh)

    nwi, nwh = len(wih_chunks), len(whh_chunks)
    rz_n = 4 * (nwh + nwi)
    rz_i = ni_i = nh_i = 0
    # gi first (its weights arrive first), gh accumulates on top
    for kc, (lhsT, rhs) in enumerate(wih_chunks):
        for g in (0, 1):
            for c in (0, 1):
                j = 2 * g + c
                nc.tensor.matmul(
                    ps_rz[:, 2 * j: 2 * j + 2],
                    lhsT[:, g * GP + C * c: g * GP + C * (c + 1)],
                    rhs, start=(rz_i == 0), stop=(rz_i == rz_n - 1))
                rz_i += 1
        for c in (0, 1):
            nc.tensor.matmul(
                ps_ni[:, 2 * c: 2 * c + 2],
                lhsT[:, 2 * GP + C * c: 2 * GP + C * (c + 1)],
                rhs, start=(ni_i == 0), stop=(ni_i == 2 * nwi - 1))
            ni_i += 1
    for kc, (lhsT, rhs) in enumerate(whh_chunks):
        for g in (0, 1):
            for c in (0, 1):
                j = 2 * g + c
                nc.tensor.matmul(
                    ps_rz[:, 2 * j: 2 * j + 2],
                    lhsT[:, g * GP + C * c: g * GP + C * (c + 1)],
                    rhs, start=(rz_i == 0), stop=(rz_i == rz_n - 1))
                rz_i += 1
        for c in (0, 1):
            nc.tensor.matmul(
                ps_nh[:, 2 * c: 2 * c + 2],
                lhsT[:, 2 * GP + C * c: 2 * GP + C * (c + 1)],
                rhs, start=(nh_i == 0), stop=(nh_i == 2 * nwh - 1))
            nh_i += 1

    # r,z = sigmoid(rz sums); n = tanh(i_n + r*h_n); h' = n + z*(h-n)
    rz = ab.tile([C, 8], F32, tag=f"{name}_rz")
    nc.scalar.activation(rz, ps_rz, AF.Sigmoid)
    t1 = ab.tile([C, 4], F32, tag=f"{name}_t1")
    nc.vector.tensor_mul(t1, rz[:, 0:4], ps_nh)
    nc.vector.tensor_add(t1, t1, ps_ni)
    n_sb = ab.tile([C, 4], F32, tag=f"{name}_n")
    nc.scalar.activation(n_sb, t1, AF.Tanh)
    d = ab.tile([C, 4], F32, tag=f"{name}_d")
    nc.vector.tensor_sub(d, h_sb.bitcast(F32), n_sb)
    nc.vector.tensor_mul(d, d, rz[:, 4:8])
    hp = ab.tile([C, 4], F32R, tag=f"{name}_hp")
    nc.vector.tensor_add(hp, n_sb, d)
    return hp


def _build():
    nc = bacc.Bacc("TRN2", num_devices=1, num_swdge_queues=4)

    d_a = nc.dram_tensor("mega_a", [128, A_F], F32R, kind="ExternalInput").ap()
    d_b = nc.dram_tensor("mega_b", [C, B_F], F32R, kind="ExternalInput").ap()
    d_c = nc.dram_tensor("mega_c", [15, C_F], F32R, kind="ExternalInput").ap()
    d_out = nc.dram_tensor("out", [1, NO], F32, kind="ExternalOutput").ap()

    with tile.TileContext(nc) as tc:
        with (
            tc.tile_pool(name="wp", bufs=1) as wp,
            tc.tile_pool(name="ab", bufs=1) as ab,
            tc.tile_pool(name="pp", bufs=1, space="PSUM") as pp,
        ):
            # ACT table warmup
            warm = ab.tile([1, 1], F32, tag="warm")
            nc.vector.memset(warm, 0.0)
            warm2 = ab.tile([1, 1], F32, tag="warm2")
            nc.scalar.activation(warm2, warm, AF.Sigmoid)
            nc.scalar.activation(warm2, warm2, AF.Tanh)

            # --- DMAs (SWDGE), priority order ---
            mc = wp.tile([15, C_F], F32R, tag="mc")
            nc.sync.dma_start(mc, d_c)        # tiny; separate HWDGE ring
            ma = wp.tile([128, A_F], F32R, tag="ma")
            nc.gpsimd.dma_start(ma, d_a)
            mb = wp.tile([C, B_F], F32R, tag="mb")
            nc.gpsimd.dma_start(mb[:, 0:B_WIH1], d_b[:, 0:B_WIH1])
            nc.gpsimd.dma_start(mb[:, B_WIH1:B_F], d_b[:, B_WIH1:B_F])

            # --- PE warmup: dummy fp32 matmuls (~3.5us of HAM activity) ---
            wz = ab.tile([128, 128], F32, tag="wz")
            nc.vector.memset(wz, 0.0)
            ps_w = pp.tile([2, 128], F32, tag="pw")
            for i in range(8):
                nc.tensor.matmul(ps_w, wz[:, 0:2], wz,
                                 start=(i == 0), stop=(i == 7))

            # --- layer 1: l1 = relu(W1 @ x + b1), [128,10] paired cols ---
            x2 = mc[:, H1 + 1:H1 + 3]
            ps_l1 = pp.tile([128, 8], F32, tag="p0")
            for c in range(4):
                nc.tensor.matmul(ps_l1[:, 2 * c:2 * c + 2],
                                 mc[:, c * 128:(c + 1) * 128], x2,
                                 start=(c == 0), stop=(c == 3))
            ps_l1b = pp.tile([49, 2], F32, tag="p5")
            nc.tensor.matmul(ps_l1b, mc[:, 512:561], x2, start=True, stop=True)
            l1_sb = ab.tile([128, 10], F32R, tag="l1")
            nc.vector.tensor_scalar_max(l1_sb[:, 0:8], ps_l1, 0.0)
            nc.vector.tensor_scalar_max(l1_sb[0:49, 8:10], ps_l1b, 0.0)

            h0_sb = mb[:, B_H0:B_H0 + 4]
            h1_sb = mb[:, B_H1:B_H1 + 4]

            # --- GRU 0 ---
            wih0_chunks = [
                (ma[:, c * M3:(c + 1) * M3], l1_sb[:, 2 * c:2 * c + 2])
                for c in range(4)
            ] + [
                (mb[0:49, B_WIH0C4:B_WIH0C4 + M3], l1_sb[0:49, 8:10])
            ]
            whh0_chunks = [
                (mb[:, B_WHH0 + c * M3: B_WHH0 + (c + 1) * M3],
                 h0_sb[:, 2 * c:2 * c + 2])
                for c in range(2)
            ]
            hp0 = _emit_gru(nc, pp, ab, "g0", wih0_chunks, whh0_chunks, h0_sb,
                            "p1", "p2", "p3")

            # --- GRU 1 ---
            wih1_chunks = [
                (mb[:, B_WIH1 + c * M3: B_WIH1 + (c + 1) * M3],
                 hp0[:, 2 * c:2 * c + 2])
                for c in range(2)
            ]
            whh1_chunks = [
                (mb[:, B_WHH1 + c * M3: B_WHH1 + (c + 1) * M3],
                 h1_sb[:, 2 * c:2 * c + 2])
                for c in range(2)
            ]
            hp1 = _emit_gru(nc, pp, ab, "g1", wih1_chunks, whh1_chunks, h1_sb,
                            "p0", "p1", "p2")

            # --- l2 ---
            ps_a = pp.tile([H2 + 1, 2], F32, tag="p3")
            for c in range(2):
                nc.tensor.matmul(
                    ps_a, mb[:, B_W2A + c * (H2 + 1): B_W2A + (c + 1) * (H2 + 1)],
                    hp1[:, 2 * c:2 * c + 2], start=(c == 0), stop=(c == 1))
            l2h = ab.tile([H2 + 1, 2], F32R, tag="l2h")
            nc.vector.tensor_scalar_max(l2h, ps_a, 0.0)
            ps_o = pp.tile([1, NO], F32, tag="p4")
            nc.tensor.matmul(ps_o, l2h[:, 0:1],
                             mb[0:H2 + 1, B_W2B:B_W2B + NO],
                             start=True, stop=True)
            out_sb = ab.tile([1, NO], F32, tag="out_sb")
            nc.vector.tensor_copy(out_sb, ps_o)
            nc.sync.dma_start(d_out, out_sb)

    nc.compile()
    return nc


def _get_nc():
    global _BUILT
    if _BUILT is None:
        _BUILT = _build()
    return _BUILT


def _gate_pack(W, b, z_pad_bias=0.0):
    """W:(435,K), b:(435,) -> (K+1, 438): W.T + bias row, per-gate 146-col
    blocks (zero pad col). z_pad_bias=100 on the ih matrix makes the h'
    garbage slot compute to exactly 1.0."""
    K = W.shape[1]
    full = np.concatenate([W.T, b[None, :]], axis=0).astype(np.float32)
    out = np.zeros((K + 1, M3), np.float32)
    for g in range(3):
        out[:, g * GP: g * GP + G] = full[:, g * G: (g + 1) * G]
    out[K, GP + G] = z_pad_bias
    return out


def pack_inputs(inputs):
    f = lambda a: np.asarray(a, np.float32)
    wih0 = _gate_pack(f(inputs["Wih0"]), f(inputs["bih0"]), 100.0)  # (561, 438)
    ma = np.zeros((128, A_F), np.float32)
    for c in range(4):
        ma[:, c * M3:(c + 1) * M3] = wih0[c * 128:(c + 1) * 128, :]

    mb = np.zeros((C, B_F), np.float32)
    hn = f(inputs["hn"])
    for col, h in ((B_H0, hn[0]), (B_H1, hn[1])):
        v = np.append(h, np.float32(1.0)).reshape(2, C).T  # [73,2]
        mb[:, col:col + 4] = v[:, [0, 0, 1, 1]]            # paired
    mb[0:49, B_WIH0C4:B_WIH0C4 + M3] = wih0[512:561, :]
    for col, W, b, zb in (
        (B_WHH0, inputs["Whh0"], inputs["bhh0"], 0.0),
        (B_WIH1, inputs["Wih1"], inputs["bih1"], 100.0),
        (B_WHH1, inputs["Whh1"], inputs["bhh1"], 0.0),
    ):
        wt = _gate_pack(f(W), f(b), zb)                    # (146, 438)
        mb[:, col:col + M3] = wt[0:C, :]
        mb[:, col + M3:col + 2 * M3] = wt[C:2 * C, :]
    w2a = np.zeros((2 * C, H2 + 1), np.float32)
    w2a[0:G + 1, 0:H2] = np.concatenate(
        [f(inputs["W2a"]).T, f(inputs["b2a"])[None, :]], axis=0)
    w2a[G, H2] = 1.0                 # unit col -> l2h slot computes to 1.0
    mb[:, B_W2A:B_W2A + H2 + 1] = w2a[0:C, :]
    mb[:, B_W2A + H2 + 1:B_W2A + 2 * (H2 + 1)] = w2a[C:2 * C, :]
    w2b = np.concatenate([f(inputs["W2b"]).T, f(inputs["b2b"])[None, :]], axis=0)
    mb[0:H2 + 1, B_W2B:B_W2B + NO] = w2b

    mc = np.zeros((15, C_F), np.float32)
    mc[:, 0:H1] = np.concatenate(
        [f(inputs["W1"]).T, f(inputs["b1"])[None, :]], axis=0)
    mc[14, H1] = 1.0                 # unit col -> l1 slot computes to 1.0
    x_ext = np.concatenate([
        f(inputs["state_inno"]), f(inputs["obs_inno"]),
        f(inputs["diff_state"]), f(inputs["diff_obs"]), [np.float32(1.0)],
    ])
    mc[:, H1 + 1] = x_ext
    mc[:, H1 + 2] = x_ext
    return {"mega_a": ma, "mega_b": mb, "mega_c": mc}


def kernel(**inputs):
    nc = _get_nc()
    in_map = pack_inputs(inputs)
    res = bass_utils.run_bass_kernel_spmd(nc, [in_map], core_ids=[0], trace=TRACE)
    kernel.last_result = res
    return np.asarray(res.results[0]["out"], np.float32).reshape(X_DIM, Y_DIM)
